# revision 3
# baseline (speedup 1.0000x reference)
"""Trainium2 Bass kernel for AdaptiveWaveletLayerSparse (GAT-style sparse
attention message passing, HOP=3) distributed over 8 NeuronCores.

Sharding: data-parallel over batch B=8 -> core i handles batch i.
Edge tables (int16, wrapped for SWDGE) are replicated to all cores.

Edges are sorted by destination on the host and padded per 32-node group
to a multiple of 128 slots. Node table rows are t-major fp8-e4m3 messages
(1536B) with bf16 f1/f2 riders (96B) in a 1632B row. Per hop, per
128-edge chunk the e-weighting is folded into the PE stationary:
  edge phase : dma_gather rows by src; expand f1[dst] per edge with a
               one-hot [32,128] matmul; e = exp(lrelu(f1d + f2s)) on
               ACT (written twice: plain + pair-duplicated e2); DVE
               builds the valued stationary stv[t] = e[:,t] (x) onehot
               with a packed-pair broadcast (2x mode); PE streams the
               raw fp8 x_t columns through stv[t] (24 matmuls of 64
               cols) + one 72-col bf16 rider matmul ([e|e*f1|e*f2])
               accumulating [y | den | f1agg | f2agg] in PSUM.
  node phase : y *= 1/den; unscaled wavelet accumulator (one
               scalar_tensor_tensor per hop); next table fp8 via ACT
               copy-with-scale; last hop projects with W (bf16) via PE
               transpose + blockdiag matmul.
"""

import os
import hashlib
import numpy as np

B, N, T, C, E, HOP = 8, 4096, 24, 64, 131072, 3
TC = T * C                   # 1536
RB = 1792                    # fp8 row bytes: x t-major 1536 | f1 48 | f2 48
                             # | pad (dma_gather elem size must be 256-mult)
F1B, F2B = 1536, 1584        # byte offsets of bf16 riders
OCOLS = TC + 3 * T           # psum cols per group: y | den | f1agg | f2agg
NEG_SLOPE = 0.2
GN = 32                      # nodes per reduce group
NGRP = N // GN               # 128
NBLK = N // 128              # 32
POOL_STV = 6                 # every POOL_STV-th stv built on gpsimd
NSTV = 14                    # rotating stv buffer count

_CACHE = {}


def _build_graph(c0, c1, c2, ch0, ch1, nhop, nblks, offs, cnts):
    import concourse.bass as bass
    import concourse.bacc as bacc
    import concourse.tile as tile
    import concourse.mybir as mybir

    f32 = mybir.dt.float32
    bf16 = mybir.dt.bfloat16
    fp8 = mybir.dt.float8e4
    i16 = mybir.dt.int16
    ALU = mybir.AluOpType
    AF = mybir.ActivationFunctionType

    SL = int(offs[-1])
    nbmax = int(np.max(nblks))

    # Wavelet recurrence, unscaled accumulator (see reference.py):
    #   y_h = mp(in_h); in_{h+1} = s_h * y_h
    #   gp_h = gp_{h-1} + (1-c2)*D_h/c2^h * y_h ;  gp_-1 = 0
    #   out = c2^(nhop-1) * gp_last + ax * x
    A_ = c0 * (2.0 * c1 - 1.0)
    D = [c1, c1 + (1.0 - c1) * (ch0 - 1.0), c1 + (1.0 - c1) * (ch1 - 1.0)]
    s = [1.0, ch0, ch1]
    coefU = [(1.0 - c2) * D[h] / (c2 ** h) for h in range(3)]
    cs = c2 ** (nhop - 1) if nhop > 0 else 1.0
    ax = c2 ** nhop
    for k in range(nhop):
        ax += (c2 ** k) * (1.0 - c2) * A_

    nc = bacc.Bacc(None, target_bir_lowering=False)

    x_in = nc.dram_tensor("x", [N, TC], bf16, kind="ExternalInput")
    srcw_d = nc.dram_tensor("srcw", [128, SL // 16], i16, kind="ExternalInput")
    sblob_d = nc.dram_tensor("sblob", [128, SL // 4], bf16,
                             kind="ExternalInput")
    stblob_d = nc.dram_tensor("stblob", [32, SL], fp8, kind="ExternalInput")
    a1r_d = nc.dram_tensor("a1r", [128, C], f32, kind="ExternalInput")
    a2r_d = nc.dram_tensor("a2r", [128, C], f32, kind="ExternalInput")
    wpad_d = nc.dram_tensor("wpad", [128, 128], bf16, kind="ExternalInput")
    br_d = nc.dram_tensor("br", [128, C], f32, kind="ExternalInput")
    id_d = nc.dram_tensor("ident", [128, 128], bf16, kind="ExternalInput")
    out_d = nc.dram_tensor("out", [N, TC], f32, kind="ExternalOutput")

    tab = [nc.dram_tensor(f"tab{i}", [N + 1, RB], fp8) for i in range(2)]
    f1tab = [nc.dram_tensor(f"f1tab{i}", [N, T], bf16) for i in range(2)]
    fptab = nc.dram_tensor("fptab", [N, TC], bf16)

    with tile.TileContext(nc) as tc:
        with (
            nc.allow_low_precision(
                reason="bf16 wavelet accumulation validated vs reference"),
            tc.tile_pool(name="cpool", bufs=1) as cpool,
            tc.tile_pool(name="gpool", bufs=5) as gpool,
            tc.tile_pool(name="vpool", bufs=3) as vpool,
            tc.tile_pool(name="wpool", bufs=1) as wpool,
            tc.tile_pool(name="spool", bufs=3) as spool,
            tc.tile_pool(name="ipool", bufs=3) as ipool,
            tc.tile_pool(name="npool", bufs=2) as npool,
            tc.tile_pool(name="qpool", bufs=1) as qpool,
            tc.tile_pool(name="ppool", bufs=1, space="PSUM") as ppool,
            tc.tile_pool(name="fpool", bufs=2, space="PSUM") as fpool,
        ):
            # ---- constants ----
            a1sb = cpool.tile([128, C], f32, name="a1sb")
            a2sb = cpool.tile([128, C], f32, name="a2sb")
            wpsb = cpool.tile([128, 128], bf16, name="wpsb")
            brsb = cpool.tile([128, C], f32, name="brsb")
            idsb = cpool.tile([128, 128], bf16, name="idsb")
            nc.sync.dma_start(a1sb[:, :], a1r_d[:, :])
            nc.sync.dma_start(a2sb[:, :], a2r_d[:, :])
            nc.sync.dma_start(wpsb[:, :], wpad_d[:, :])
            nc.sync.dma_start(brsb[:, :], br_d[:, :])
            nc.sync.dma_start(idsb[:, :], id_d[:, :])
            zr = cpool.tile([1, RB], fp8, name="zr")
            nc.vector.memset(zr[:, :], 0.0)
            for i in range(2):
                nc.sync.dma_start(tab[i][N:N + 1, :], zr[:, :])

            a1b = a1sb.unsqueeze(1).broadcast_to([128, T, C])
            a2b = a2sb.unsqueeze(1).broadcast_to([128, T, C])

            def project(fpT, blk):
                """out[blk] = fpT @ blockdiag(W,W) + b via T -> mm -> T.
                fpT: [128, TC] bf16, t-major."""
                ost = npool.tile([128, TC], f32, name="ost", tag="ost")
                for k in range(TC // 128):
                    p1 = ppool.tile([128, 128], bf16, name="p1", tag=f"pj{k % 2}")
                    nc.tensor.transpose(
                        p1[:, :], fpT[:, k * 128:(k + 1) * 128], idsb[:, :])
                    s1 = spool.tile([128, 128], bf16, name="s1")
                    nc.scalar.copy(s1[:, :], p1[:, :])
                    p2 = ppool.tile([128, 128], f32, name="p2", tag=f"pj{k % 2}")
                    nc.tensor.matmul(
                        p2[:, :], wpsb[:, :], s1[:, :], start=True, stop=True)
                    s2 = spool.tile([128, 128], bf16, name="s2")
                    nc.scalar.copy(s2[:, :], p2[:, :])
                    p3 = ppool.tile([128, 128], bf16, name="p3", tag=f"pj{k % 2}")
                    nc.tensor.transpose(p3[:, :], s2[:, :], idsb[:, :])
                    ov = ost[:, k * 128:(k + 1) * 128].rearrange(
                        "p (a c) -> p a c", c=C)
                    p3v = p3.rearrange("p (a c) -> p a c", c=C)
                    bb = brsb.unsqueeze(1).broadcast_to([128, 2, C])
                    nc.vector.tensor_tensor(ov, p3v, bb, ALU.add)
                nc.scalar.dma_start(
                    out_d[blk * 128:(blk + 1) * 128, :], ost[:, :])

            if nhop == 0:
                for blk in range(NBLK):
                    xT = npool.tile([128, TC], bf16, name="xT", tag="fpT")
                    nc.sync.dma_start(
                        xT[:, :], x_in[blk * 128:(blk + 1) * 128, :])
                    project(xT, blk)

            # ---- prologue: tab0 = [fp8(x) tmaj | f1 | f2], f1tab0 ----
            for blk in range(NBLK if nhop > 0 else 0):
                xbh = npool.tile([128, TC], bf16, name="xbh", tag="xbh")
                nc.sync.dma_start(
                    xbh[:, :], x_in[blk * 128:(blk + 1) * 128, :])
                tb = npool.tile([128, RB], fp8, name="tb", tag="tb2")
                nc.scalar.copy(tb[:, 0:TC], xbh[:, :])
                xtv = xbh.rearrange("p (t c) -> p t c", c=C)
                tmp = qpool.tile([128, T, C], bf16, name="tmp", tag="tmp")
                nc.vector.tensor_tensor(tmp[:, :, :], xtv, a1b, ALU.mult)
                fr = npool.tile([128, 2, T], f32, name="fr", tag="fr")
                nc.vector.tensor_reduce(
                    fr[:, 0, :], tmp[:, :, :], mybir.AxisListType.X, ALU.add)
                tmp2 = qpool.tile([128, T, C], bf16, name="tmp2", tag="tmp2")
                nc.gpsimd.tensor_tensor(tmp2[:, :, :], xtv, a2b, ALU.mult)
                nc.vector.tensor_reduce(
                    fr[:, 1, :], tmp2[:, :, :], mybir.AxisListType.X, ALU.add)
                tbf = tb[:, F1B:F1B + 96].bitcast(bf16)
                nc.vector.tensor_copy(tbf, fr.rearrange("p x t -> p (x t)"))
                f1sc = npool.tile([128, T], bf16, name="f1sc", tag="f1sc")
                nc.vector.tensor_copy(f1sc[:, :], fr[:, 0, :])
                nc.scalar.dma_start(
                    f1tab[0][blk * 128:(blk + 1) * 128, :], f1sc[:, :])
                nc.scalar.dma_start(
                    tab[0][blk * 128:(blk + 1) * 128, 0:F2B + 48],
                    tb[:, 0:F2B + 48])

            for h in range(nhop):
                tcur = tab[h % 2]
                tnext = tab[(h + 1) % 2]
                f1cur = f1tab[h % 2]
                f1next = f1tab[(h + 1) % 2]
                blkstate = {}
                pending_stores = []
                pending_node = []
                stv_ctr = [0]
                gctr = [0]

                def emit_loads(g, f1cur=f1cur):
                    """Small SP loads for group g (2 groups ahead)."""
                    blk, gg = divmod(g, 4)
                    nb = int(nblks[g])
                    off = int(offs[g])
                    boff = int(offs[blk * 4])
                    goff = off - boff
                    sl = nb * 128
                    if gg == 0:
                        bend = int(offs[blk * 4 + 4])
                        St = spool.tile([128, (bend - boff) // 4], bf16,
                                        name="St", tag="St")
                        nc.sync.dma_start(
                            St[:, :], sblob_d[:, boff // 4:bend // 4])
                        blkstate[blk] = St
                    si = ipool.tile([128, sl // 16], i16, name="si", tag="si")
                    nc.sync.dma_start(
                        si[:, :], srcw_d[:, off // 16:(off + sl) // 16])
                    f1g = ipool.tile([32, T], bf16, name="f1g", tag="f1g")
                    nc.sync.dma_start(
                        f1g[:, :], f1cur[g * GN:(g + 1) * GN, :])
                    stb = spool.tile([32, sl], fp8, name="stb", tag="stb")
                    nc.sync.dma_start(stb[:, :], stblob_d[:, off:off + sl])
                    return (g, nb, goff, si, f1g, stb)

                def emit_gather(state, tcur=tcur):
                    """fp8 row gather (Pool SWDGE) for group g (1 ahead).
                    After a 5-gather warmup (fills all rotating G buffers
                    with valid fp8/bf16 bytes), pad slots past the group's
                    real edge count are not gathered; stale buffer bytes
                    are masked by zero stv/rider stationaries."""
                    g, nb, goff, si, f1g, stb = state
                    G = gpool.tile([128, nb, RB], fp8, name="G", tag="G")
                    for b0 in range(0, nb, 8):
                        bn = min(8, nb - b0)
                        ssl = bn * 128
                        isl = si[:, b0 * 8:b0 * 8 + ssl // 16]
                        nc.gpsimd.dma_gather(
                            G[:, b0:b0 + bn, :], tcur[:, 0:RB], isl,
                            ssl, ssl, RB, elem_step=RB)
                    return (g, nb, goff, G, f1g, stb)

                def emit_prep(state, h=h):
                    g, nb, goff, G, f1g, stb = state
                    blk, gg = divmod(g, 4)
                    St = blkstate[blk]

                    # f1[dst] per edge: fe = stb_j^T @ f1g (PE expand)
                    fe = fpool.tile([128, nbmax * T], f32, name="fe",
                                    tag="fe")
                    for j in range(nb):
                        nc.tensor.matmul(
                            fe[:, j * T:(j + 1) * T],
                            stb[:, j * 128:(j + 1) * 128],
                            f1g[:, :], start=True, stop=True)

                    Gf = G[:, :, F1B:F1B + 96].bitcast(bf16)
                    z = vpool.tile([128, nb, T], bf16, name="z", tag="z")
                    nc.vector.tensor_tensor(
                        z[:, :, :],
                        fe[:, 0:nb * T].rearrange("p (a t) -> p a t", t=T),
                        Gf.rearrange("p a (x t) -> p a x t", t=T)
                        [:, :, 1, :], ALU.add)
                    zl = vpool.tile([128, nb, T], bf16, name="zl", tag="zl")
                    nc.vector.scalar_tensor_tensor(
                        zl[:, :, :], z[:, :, :], NEG_SLOPE, z[:, :, :],
                        ALU.mult, ALU.max)
                    # e plain (for riders) + pair-duplicated e2 (for stv)
                    e = vpool.tile([128, nb, T], bf16, name="e", tag="e")
                    nc.scalar.activation(e[:, :, :], zl[:, :, :], AF.Exp)
                    e2 = vpool.tile([128, nb, T, 2], bf16, name="e2",
                                    tag="e2")
                    nc.scalar.activation(
                        e2[:, :, :, :],
                        zl.unsqueeze(3).broadcast_to([128, nb, T, 2]),
                        AF.Exp)
                    # rider moving blob [e | e*f1 | e*f2]
                    erid = vpool.tile([128, nb, 3, T], bf16, name="erid",
                                      tag="erid")
                    nc.vector.tensor_copy(erid[:, :, 0, :], e[:, :, :])
                    nc.vector.tensor_tensor(
                        erid[:, :, 1:3, :],
                        Gf.rearrange("p a (x t) -> p a x t", t=T),
                        e.unsqueeze(2).broadcast_to([128, nb, 2, T]),
                        ALU.mult)

                    # valued stationaries stv[t] = e[:, t] (x) onehot,
                    # globally rotating buffer pool (DVE, some on gpsimd)
                    stvs = []
                    for j in range(nb):
                        stj = St[:, goff // 4 + j * GN:
                                 goff // 4 + (j + 1) * GN]
                        k = stv_ctr[0] % NSTV
                        stv_ctr[0] += 1
                        stv = wpool.tile([128, T, GN], bf16, name="stv",
                                         tag=f"stv{k}")
                        stv4 = stv.rearrange("p t (a b) -> p t a b", b=2)
                        e2j = e2[:, j, :, :].unsqueeze(2).broadcast_to(
                            [128, T, GN // 2, 2])
                        ohj = stj.rearrange("p (a b) -> p a b", b=2) \
                            .unsqueeze(1).broadcast_to([128, T, GN // 2, 2])
                        if k % POOL_STV == POOL_STV - 1:
                            nc.gpsimd.tensor_tensor(stv4, e2j, ohj, ALU.mult)
                        else:
                            nc.vector.tensor_tensor(stv4, e2j, ohj, ALU.mult)
                        stvs.append(stv)
                    return (g, nb, goff, G, stvs, erid)

                def emit_mm(state, h=h, tnext=tnext, f1next=f1next):
                    g, nb, goff, G, stvs, erid = state
                    blk, gg = divmod(g, 4)
                    if gg == 0:
                        for dst, srct in pending_stores:
                            nc.sync.dma_start(dst, srct)
                        pending_stores.clear()
                    St = blkstate[blk]
                    if gg == 0:
                        blkstate[blk, "pm"] = ppool.tile(
                            [128, OCOLS], f32, name="pm", tag="pm")
                    pm = blkstate[blk, "pm"]
                    pms = pm[32 * gg:32 * (gg + 1), :]
                    for j in range(nb):
                        stj = St[:, goff // 4 + j * GN:
                                 goff // 4 + (j + 1) * GN]
                        stv = stvs[j]
                        st = (j == 0)
                        sp = (j == nb - 1)
                        for t in range(T):
                            # start=True resets the whole 2KB PSUM bank:
                            # assert it only on the first matmul touching
                            # each bank (t=0,8,16 and the rider in bank 3)
                            nc.tensor.matmul(
                                pms[:, t * C:(t + 1) * C], stv[:, t, :],
                                G[:, j, t * C:(t + 1) * C],
                                start=st and t % 8 == 0, stop=sp,
                                tile_position=(0, 32 * gg))
                        nc.tensor.matmul(
                            pms[:, TC:OCOLS], stj,
                            erid[:, j, :, :].rearrange("p x t -> p (x t)"),
                            start=st, stop=sp,
                            tile_position=(0, 32 * gg))
                    del pms
                    if gg < 3:
                        return

                    # ---- node phase (after the block's 4th group) ----
                    del blkstate[blk]
                    del blkstate[blk, "pm"]
                    o = npool.tile([128, OCOLS], bf16, name="o", tag="o")
                    nc.scalar.copy(o[:, 0:OCOLS], pm[:, 0:OCOLS])
                    pending_node.append((blk, o))

                def emit_node(blk, o, h=h, tnext=tnext, f1next=f1next):
                    den = o[:, TC:TC + T]
                    nc.vector.tensor_scalar_max(den, den, 1e-8)
                    rec = npool.tile([128, T], bf16, name="rec", tag="rec")
                    nc.vector.reciprocal(rec[:, :], den)
                    rec2 = npool.tile([128, T, 2], bf16, name="rec2",
                                      tag="rec2")
                    nc.vector.tensor_copy(
                        rec2[:, :, :],
                        rec.unsqueeze(2).broadcast_to([128, T, 2]))
                    yv = o[:, 0:TC].rearrange(
                        "p (t a b) -> p t a b", t=T, b=2)
                    recb = rec2.unsqueeze(2).broadcast_to([128, T, C // 2, 2])
                    nc.vector.tensor_tensor(yv, yv, recb, ALU.mult)
                    # normalize rider aggregates
                    fv = o[:, TC + T:OCOLS].rearrange("p (x t) -> p x t", t=T)
                    rb2 = rec.unsqueeze(1).broadcast_to([128, 2, T])
                    nc.vector.tensor_tensor(fv, fv, rb2, ALU.mult)

                    # unscaled wavelet accumulator
                    gp = npool.tile([128, TC], bf16, name="gp", tag="gp")
                    if h == 0:
                        nc.vector.tensor_scalar_mul(
                            gp[:, :], o[:, 0:TC], coefU[0])
                    else:
                        gpb = npool.tile([128, TC], bf16, name="gpb",
                                         tag="gpb")
                        nc.sync.dma_start(
                            gpb[:, :], fptab[blk * 128:(blk + 1) * 128, :])
                        nc.vector.scalar_tensor_tensor(
                            gp[:, :], o[:, 0:TC], coefU[h], gpb[:, :],
                            ALU.mult, ALU.add)

                    if h < nhop - 1:
                        pending_stores.append(
                            (fptab[blk * 128:(blk + 1) * 128, :],
                             gp[:, :]))
                        # next table: x' = fp8(s*y) tmaj, riders = s*f*rec
                        tb2 = npool.tile([128, RB], fp8, name="tb2",
                                         tag="tb2")
                        nc.scalar.activation(
                            tb2[:, 0:TC], o[:, 0:TC], AF.Copy, scale=s[h])
                        tb2f = tb2[:, F1B:F1B + 96].bitcast(bf16)
                        nc.vector.tensor_scalar_mul(
                            tb2f, o[:, TC + T:OCOLS], s[h])
                        f1sc = npool.tile([128, T], bf16, name="f1sc",
                                          tag="f1sc")
                        nc.vector.tensor_scalar_mul(
                            f1sc[:, :], o[:, TC + T:TC + 2 * T], s[h])
                        pending_stores.append(
                            (f1next[blk * 128:(blk + 1) * 128, :],
                             f1sc[:, :]))
                        pending_stores.append(
                            (tnext[blk * 128:(blk + 1) * 128, 0:F2B + 48],
                             tb2[:, 0:F2B + 48]))
                    elif os.environ.get("KERNEL_DEBUG_O"):
                        ost = npool.tile([128, TC], f32, name="ost",
                                         tag="ost")
                        nc.vector.tensor_copy(ost[:, :], o[:, 0:TC])
                        if os.environ["KERNEL_DEBUG_O"] == "2":
                            nc.vector.tensor_copy(
                                ost[:, 0:3 * T], o[:, TC:OCOLS])
                        nc.scalar.dma_start(
                            out_d[blk * 128:(blk + 1) * 128, :], ost[:, :])
                    else:
                        xb = npool.tile([128, TC], bf16, name="xb",
                                        tag="xb")
                        nc.sync.dma_start(
                            xb[:, :], x_in[blk * 128:(blk + 1) * 128, :])
                        xpre = npool.tile([128, TC], bf16, name="xpre",
                                          tag="xpre")
                        nc.scalar.activation(
                            xpre[:, :], xb[:, :], AF.Copy, scale=ax)
                        fpT = npool.tile([128, TC], bf16, name="fpT",
                                         tag="fpT")
                        nc.vector.scalar_tensor_tensor(
                            fpT[:, :], gp[:, :], cs, xpre[:, :],
                            ALU.mult, ALU.add)
                        project(fpT, blk)

                loaded = []
                gathered = []
                prepared = []

                def step(g=None):
                    if g is not None:
                        loaded.append(emit_loads(g))
                    if loaded and (g is None or len(loaded) > 1):
                        gathered.append(emit_gather(loaded.pop(0)))
                    if gathered and (g is None or len(gathered) > 2):
                        prepared.append(emit_prep(gathered.pop(0)))
                    if prepared and (g is None or len(prepared) > 1):
                        emit_mm(prepared.pop(0))
                        while pending_node:
                            emit_node(*pending_node.pop(0))

                for g in range(NGRP):
                    step(g)
                while loaded or gathered or prepared:
                    step()
                while pending_node:
                    emit_node(*pending_node.pop(0))
                for dst, srct in pending_stores:
                    nc.sync.dma_start(dst, srct)
                pending_stores.clear()
    nc.finalize()
    return nc


def _wrap_idx(arr):
    """int array [SL] -> int16 [128, SL//16] wrapped (j -> [j%16, j//16]),
    replicated for the 8 gpsimd cores."""
    w = arr.astype(np.int16).reshape(-1, 16).T.copy()
    return np.ascontiguousarray(np.tile(w, (8, 1)))


def _prep_edges(dst, src):
    """Sort by dst, pad per 32-node group to multiples of 128 slots, build
    slot index arrays and the one-hot S blobs."""
    perm = np.argsort(dst, kind="stable")
    dsts, srcs = dst[perm], src[perm]
    bounds = np.searchsorted(dsts, np.arange(0, N + 1, GN))
    cnts = np.diff(bounds)
    nblks = np.maximum(np.ceil(cnts / 128).astype(np.int64), 1)
    offs = np.concatenate([[0], np.cumsum(nblks * 128)])
    SL = int(offs[-1])
    src_slots = np.full(SL, N, np.int64)  # pad -> sentinel zero row
    sblob = np.zeros((128, SL // 4), np.float32)
    stblob = np.zeros((32, SL), np.float32)
    for g in range(NGRP):
        lo, hi = int(bounds[g]), int(bounds[g + 1])
        cnt = hi - lo
        if cnt == 0:
            continue
        o0 = int(offs[g])
        src_slots[o0:o0 + cnt] = srcs[lo:hi]
        slots = np.arange(o0, o0 + cnt)
        bi = slots // 128
        r = slots % 128
        cloc = dsts[lo:hi] - g * GN
        sblob[r, bi * GN + cloc] = 1.0
        stblob[cloc, slots] = 1.0
    return nblks, offs, _wrap_idx(src_slots), sblob, stblob, cnts


def kernel(**inputs):
    from concourse.bass_utils import run_bass_kernel_spmd
    import ml_dtypes

    x = np.asarray(inputs["x"], dtype=np.float32)          # [B,N,T,C]
    edge_index = np.asarray(inputs["edge_index"])          # [2,E] int
    a = np.asarray(inputs["a"], dtype=np.float32)          # [2C,1]
    temp = np.asarray(inputs["temp"], dtype=np.float32)    # [4]
    cheb = np.asarray(inputs["cheb"], dtype=np.float32)    # [4]
    W = np.asarray(inputs["W"], dtype=np.float32)          # [C,C]
    b = np.asarray(inputs["b"], dtype=np.float32)          # [C]

    coe = 1.0 / (1.0 + np.exp(-temp.astype(np.float64)))
    chc = 1.0 / (1.0 + np.exp(-cheb.astype(np.float64)))

    dst = edge_index[0].astype(np.int64)
    src = edge_index[1].astype(np.int64)
    nblks, offs, srcw, sblob, stblob, cnts = _prep_edges(dst, src)

    nhop = int(os.environ.get("KERNEL_HOPS", str(HOP)))
    ehash = hashlib.md5(edge_index.tobytes()).hexdigest()
    key = (round(float(coe[0]), 12), round(float(coe[1]), 12),
           round(float(coe[2]), 12), round(float(chc[0]), 12),
           round(float(chc[1]), 12), nhop, ehash, "v3")
    if key not in _CACHE:
        _CACHE[key] = _build_graph(
            float(coe[0]), float(coe[1]), float(coe[2]),
            float(chc[0]), float(chc[1]), nhop, nblks, offs, cnts)
    nc = _CACHE[key]

    bfd = ml_dtypes.bfloat16
    a1r = np.ascontiguousarray(np.tile(a[:C, 0], (128, 1)).astype(np.float32))
    a2r = np.ascontiguousarray(np.tile(a[C:, 0], (128, 1)).astype(np.float32))
    wpad = np.zeros((128, 128), dtype=np.float32)
    wpad[:C, :C] = W
    wpad[C:, C:] = W
    br = np.ascontiguousarray(np.tile(b, (128, 1)).astype(np.float32))
    ident = np.eye(128, dtype=np.float32)

    in_maps = []
    for i in range(B):
        in_maps.append({
            "x": np.ascontiguousarray(x[i].reshape(N, TC)).astype(bfd),
            "srcw": srcw, "sblob": sblob.astype(bfd),
            "stblob": stblob.astype(ml_dtypes.float8_e4m3),
            "a1r": a1r, "a2r": a2r, "wpad": wpad.astype(bfd), "br": br,
            "ident": ident.astype(bfd),
        })

    trace = bool(int(os.environ.get("KERNEL_TRACE", "0")))
    res = run_bass_kernel_spmd(nc, in_maps, core_ids=list(range(B)),
                               trace=trace)
    if trace and res.exec_time_ns is not None:
        print(f"HW exec time: {res.exec_time_ns} ns")
        kernel.last_exec_time_ns = res.exec_time_ns
        kernel.last_profile = res
    out = np.stack([res.results[i]["out"] for i in range(B)])
    return out.reshape(B, N, T, C)
